# revision 17
# baseline (speedup 1.0000x reference)
"""MoE gate (softmax + top-8 + renormalize) Trainium2 Bass kernel.

Problem: hidden_states [4, 4096, 2048] f32, weight [64, 2048] f32.
  logits = x @ W.T            [16384, 64]
  scores = softmax(logits)
  topk_w, topk_idx = top_k(scores, 8);  topk_w /= topk_w.sum(-1)

Key identities used:
  - top-8 indices of softmax(logits) == top-8 indices of logits
  - renormalized top-8 softmax probs == softmax over just the top-8 logits
    (the global softmax denominator cancels), so the full [T,64] softmax is
    never materialized.

Sharding: tokens split 2048-per-core across 8 NeuronCores; weight replicated.
The token shard of x is transposed on the HOST (numpy) so the device reads
x^T with H on partitions — the layout the PE's contraction needs — at full
contiguous DMA bandwidth. No on-device transposes of the big tensor.

Per core device program:
  - load W^T [2048, 64] once (512 KB)
  - for each quarter of 512 tokens:
      for each of 16 H-tiles: DMA x^T panel [128, 512] (256 KB, contiguous),
      4 matmuls (lhsT = x^T block [128h,128t], rhs = W^T tile [128h,64e])
      accumulating logits [128t, 64e] in PSUM over the 16 H-tiles
  - epilogue per 128-token tile: copy PSUM->SBUF, hardware top-8
    (InstMax + InstMaxIndex), exp (ACT, with per-partition -max bias and
    fused sum), reciprocal, scale -> weights; stage results
  - two output DMAs: weights [2048, 8] f32, indices [2048, 8] u32
"""

import sys

if "/opt/trn_rl_repo" not in sys.path:
    sys.path.insert(0, "/opt/trn_rl_repo")

import numpy as np

N_CORES = 8
T_TOTAL = 16384
T_CORE = T_TOTAL // N_CORES   # 2048 tokens per core
H = 2048
E = 64
TOP_K = 8

HT = H // 128                 # 16 contraction tiles
NQ = 4                        # token quarters per core
TQ = T_CORE // NQ             # 512 tokens per quarter
JT = TQ // 128                # 4 token-tiles of 128 per quarter

_cached = {}


def _build_program():
    import concourse.bass as bass
    import concourse.tile as tile
    import concourse.tile_sem_assignment as tsa
    from concourse import mybir

    # Walrus allows only ONE sync-wait command on a (self-loading fp32)
    # Matmult. Tile round-robins HWDGE DMA completions across 8 sem lanes,
    # which can leave a matmul waiting on two lanes at once. All our DMAs
    # are issued from the single SP HWDGE ring (FIFO completion order), so
    # collapsing to one lane is lossless and every PE wait becomes a single
    # monotonic sem-ge condition.
    tsa.NUM_HWDGE_SEMS = 1

    f32 = mybir.dt.float32
    u32 = mybir.dt.uint32

    nc = bass.Bass()
    xt = nc.dram_tensor("xt", [H, T_CORE], f32, kind="ExternalInput")
    wt = nc.dram_tensor("wt", [H, E], f32, kind="ExternalInput")
    out_w = nc.dram_tensor("out_w", [T_CORE, TOP_K], f32, kind="ExternalOutput")
    out_i = nc.dram_tensor("out_i", [T_CORE, TOP_K], u32, kind="ExternalOutput")

    with tile.TileContext(nc) as tc:
        with (
            tc.tile_pool(name="wpool", bufs=1) as wpool,
            # One buffer per panel DMA (64 x 2KB/partition = 128KB/partition):
            # no SBUF slot reuse, so panel DMAs carry zero sync waits (the
            # HWDGE DMA descriptor, like the fp32 matmul, supports only one).
            tc.tile_pool(name="xpool", bufs=NQ * HT) as xpool,
            tc.tile_pool(name="psum", bufs=7, space="PSUM") as psum,
            tc.tile_pool(name="dummy", bufs=1, space="PSUM") as dummy_pool,
            # One buffer per token-tile: epilogue tiles are tiny and slot
            # reuse would add second sync-waits (HW limit: one per inst).
            tc.tile_pool(name="epi", bufs=NQ * JT) as epi,
            tc.tile_pool(name="stage", bufs=1) as stage,
        ):
            wt_sb = wpool.tile([128, HT, E], f32)
            nc.sync.dma_start(wt_sb[:], wt.rearrange("(a p) e -> p a e", p=128))

            stage_w = stage.tile([128, T_CORE // 128, TOP_K], f32)
            stage_i = stage.tile([128, T_CORE // 128, TOP_K], u32)

            # Wait-collector target: walrus allows only one sync-wait per
            # fp32 matmul, so at each quarter boundary a throwaway 1x1
            # matmul absorbs the panel-DMA wait; the real matmuls then only
            # carry the (single, monotonic) DVE psum-release wait.
            dummy_ps = dummy_pool.tile([1, 1], f32)

            last_per_engine = {}
            for q in range(NQ):
                ps_tiles = [
                    psum.tile([128, E], f32, tag="ps", name=f"ps_{q}_{j}")
                    for j in range(JT)
                ]
                for h in range(HT):
                    xp = xpool.tile([128, TQ], f32)
                    last_per_engine["dma_in"] = nc.sync.dma_start(
                        xp[:], xt[128 * h : 128 * (h + 1), TQ * q : TQ * (q + 1)]
                    )
                    dmy = None
                    if h == 0:
                        dmy = nc.tensor.matmul(
                            dummy_ps[:], xp[0:1, 0:1], xp[0:1, 0:1],
                            start=True, stop=True,
                        )
                    for j in range(JT):
                        mm = nc.tensor.matmul(
                            ps_tiles[j][:],
                            xp[:, bass.ts(j, 128)],
                            wt_sb[:, h, :],
                            start=(h == 0),
                            stop=(h == HT - 1),
                        )
                        last_per_engine["pe"] = mm
                        if dmy is not None:
                            tile.add_dep_helper(
                                mm.ins, dmy.ins, sync=False,
                                reason="order real MMs after wait-collector",
                            )
                for j in range(JT):
                    tt = q * JT + j
                    s = epi.tile([128, E], f32)
                    nc.vector.tensor_copy(s[:], ps_tiles[j][:])
                    vals = epi.tile([128, TOP_K], f32)
                    nc.vector.max(vals[:], s[:])
                    nc.vector.max_index(stage_i[:, tt, :], vals[:], s[:])
                    negm = epi.tile([128, 1], f32)
                    nc.vector.tensor_scalar_mul(negm[:], vals[:, 0:1], -1.0)
                    ex = epi.tile([128, TOP_K], f32)
                    ssum = epi.tile([128, 1], f32)
                    last_per_engine["act"] = nc.scalar.activation(
                        ex[:],
                        vals[:],
                        mybir.ActivationFunctionType.Exp,
                        bias=negm[:],
                        scale=1.0,
                        accum_out=ssum[:],
                    )
                    rcp = epi.tile([128, 1], f32)
                    nc.vector.reciprocal(rcp[:], ssum[:])
                    last_per_engine["dve"] = nc.vector.tensor_scalar_mul(
                        stage_w[:, tt, :], ex[:], rcp[:]
                    )

            # SWDGE path: lands on the (otherwise unused) DMASW sem lane, so
            # each output DMA carries exactly one wait (its DVE data dep)
            # instead of DVE + HWDGE-lane catch-up.
            last_per_engine["dma_w"] = nc.gpsimd.dma_start(
                out_w.rearrange("(a p) k -> p a k", p=128), stage_w[:]
            )
            last_per_engine["dma_i"] = nc.gpsimd.dma_start(
                out_i.rearrange("(a p) k -> p a k", p=128), stage_i[:]
            )

            # The kernel-tail drain on SP must catch its clock up to every
            # other proc; walrus only allows one sync-wait per instruction,
            # so stage the catch-up through single-dep SP nops first.
            for key, target in last_per_engine.items():
                nop = nc.sync.nop(hint=f"sp_catchup_{key}", nofuse=True)
                tile.add_dep_helper(
                    nop.ins, target.ins, sync=True,
                    reason=f"SP clock catch-up on {key}",
                )

    for f in nc.m.functions:
        for b in f.blocks:
            for inst in b.instructions:
                if type(inst).__name__ == "InstMatmult" and inst.sync_info:
                    assert len(inst.sync_info.on_wait) <= 1, (
                        f"{inst.name} has {len(inst.sync_info.on_wait)} waits"
                    )
    return nc


def _get_program():
    if "nc" not in _cached:
        _cached["nc"] = _build_program()
    return _cached["nc"]


def _make_in_maps(hidden_states, weight):
    x = np.asarray(hidden_states, dtype=np.float32).reshape(T_TOTAL, H)
    wt = np.ascontiguousarray(np.asarray(weight, dtype=np.float32).T)
    in_maps = []
    for i in range(N_CORES):
        xs = x[i * T_CORE : (i + 1) * T_CORE]
        in_maps.append({"xt": np.ascontiguousarray(xs.T), "wt": wt})
    return in_maps


def _gather(results):
    topk_w = np.concatenate([results[i]["out_w"] for i in range(N_CORES)], axis=0)
    topk_i = np.concatenate([results[i]["out_i"] for i in range(N_CORES)], axis=0)
    return topk_w.astype(np.float32), topk_i.astype(np.int32)


def kernel(hidden_states, weight):
    from concourse.bass_utils import run_bass_kernel_spmd

    nc = _get_program()
    in_maps = _make_in_maps(hidden_states, weight)
    res = run_bass_kernel_spmd(nc, in_maps, list(range(N_CORES)))
    return _gather(res.results)
